# revision 7
# baseline (speedup 1.0000x reference)
"""AxialAttention (MSA row attention) Trainium2 Bass kernel, 8-core SPMD.

Sharding: the s=128 MSA-row axis is split 16 rows/core across 8 cores.
Params are replicated; the pairwise attention bias is recomputed on every
core from a CPU-pre-transposed copy of `edges` (pure layout transform).

Per-core dataflow (all matmuls in float32r = full-rate TF32-like):
  LayerNorm (tokens on partitions, bn_stats)  ->  PE-transpose x_c
  qT/kT/gT = W_g.T @ x_cT   (f on partitions)     v in natural layout
  scoresT[j,i] = bias^T (identity-injected into PSUM) + kT.T@qT
  P^T = exp(scoresT)        (no max subtraction: logits bounded ~+-2)
  AV with ones-augmented V rows  -> out^T and Z in one matmul
  1/Z broadcast via K=1 ones matmul, normalize+gate on DVE
  out = gatedT.T @ Wo + bo (K=1 inject), PSUM DMA'd straight to DRAM
"""
import sys

if "/opt/trn_rl_repo" not in sys.path:
    sys.path.insert(0, "/opt/trn_rl_repo")

import numpy as np

import concourse.bass as bass
import concourse.tile as tile
from concourse import bacc, mybir
from concourse.bass_utils import run_bass_kernel_spmd

F32 = mybir.dt.float32
F32R = mybir.dt.float32r
AF = mybir.ActivationFunctionType
ALU = mybir.AluOpType

N_CORES = 8
S = 128                 # MSA rows (axial batch)
S_PER_CORE = S // N_CORES
N = 256                 # sequence positions per row
D = 256                 # node dim
HEADS = 8
DH = 64                 # head dim
DI = HEADS * DH         # 512
DE = 128                # edge dim
T_EDGE = N * N          # 65536 flattened (j,i) pairs
EDGE_CHUNK = 1024       # t' per bias-phase chunk
SCALE = DH ** -0.5


def build_nc():
    nc = bacc.Bacc("TRN2", target_bir_lowering=False, debug=False,
                   num_devices=N_CORES)

    x_in = nc.dram_tensor("x", [S_PER_CORE * N, D], F32, kind="ExternalInput").ap()
    edgesT = nc.dram_tensor("edgesT", [DE, T_EDGE], F32R, kind="ExternalInput").ap()
    wq_in = nc.dram_tensor("Wq", [D, DI], F32, kind="ExternalInput").ap()
    wkv_in = nc.dram_tensor("Wkv", [D, 2 * DI], F32, kind="ExternalInput").ap()
    wg_in = nc.dram_tensor("Wg", [D, DI], F32, kind="ExternalInput").ap()
    wo_in = nc.dram_tensor("Wo", [DI, D], F32R, kind="ExternalInput").ap()
    web_in = nc.dram_tensor("Web", [DE, 16], F32R, kind="ExternalInput").ap()
    gamma_in = nc.dram_tensor("gamma", [1, D], F32, kind="ExternalInput").ap()
    beta_in = nc.dram_tensor("beta", [1, D], F32, kind="ExternalInput").ap()
    bo_in = nc.dram_tensor("bo", [1, D], F32R, kind="ExternalInput").ap()
    bg_in = nc.dram_tensor("bg", [1, DI], F32, kind="ExternalInput").ap()
    consts = nc.dram_tensor("consts", [128, 288], F32R, kind="ExternalInput").ap()
    out_d = nc.dram_tensor("out", [S_PER_CORE * N, D], F32, kind="ExternalOutput").ap()

    with tile.TileContext(nc) as tc, nc.allow_low_precision(
        reason="float32r storage for matmul operands"
    ):
        _emit(nc, tc, x_in, edgesT, wq_in, wkv_in, wg_in, wo_in, web_in,
              gamma_in, beta_in, bo_in, bg_in, consts, out_d)
    nc.compile()
    return nc


def _emit(nc, tc, x_in, edgesT, wq_in, wkv_in, wg_in, wo_in, web_in,
          gamma_in, beta_in, bo_in, bg_in, consts, out_d):
    from contextlib import ExitStack
    ctx = ExitStack()
    const = ctx.enter_context(tc.tile_pool(name="const", bufs=1))
    work = ctx.enter_context(tc.tile_pool(name="work", bufs=2))
    small = ctx.enter_context(tc.tile_pool(name="small", bufs=4))
    edg = ctx.enter_context(tc.tile_pool(name="edg", bufs=3))
    ps = ctx.enter_context(tc.tile_pool(name="ps", bufs=8, space="PSUM"))
    dram = ctx.enter_context(tc.tile_pool(name="dram", bufs=1, space="DRAM"))

    def pst(shape, dtype=F32, name="pst"):
        return ps.tile(shape, dtype, tag="ps", name=name)

    # ---- constants / weights ----
    consts_sb = const.tile([128, 288], F32R)
    nc.sync.dma_start(consts_sb, consts)
    ident = consts_sb[:, 0:128]
    ones_1x128 = consts_sb[0:1, 128:256]
    ones_1x64 = consts_sb[0:1, 128:192]

    wq_sb = const.tile([128, 2, DI], F32)
    nc.sync.dma_start(wq_sb, wq_in.rearrange("(kt p) f -> p kt f", p=128))
    wk_sb = const.tile([128, 2, DI], F32)
    nc.sync.dma_start(wk_sb, wkv_in[:, 0:DI].rearrange("(kt p) f -> p kt f", p=128))
    wv_sb = const.tile([128, 2, DI], F32)
    nc.sync.dma_start(wv_sb, wkv_in[:, DI:2 * DI].rearrange("(kt p) f -> p kt f", p=128))
    wg_sb = const.tile([128, 2, DI], F32)
    nc.sync.dma_start(wg_sb, wg_in.rearrange("(kt p) f -> p kt f", p=128))
    wo_sb = const.tile([128, 4, D], F32R)
    nc.sync.dma_start(wo_sb, wo_in.rearrange("(kt p) f -> p kt f", p=128))
    web_sb = const.tile([128, 16], F32R)
    nc.sync.dma_start(web_sb, web_in)
    bo_sb = const.tile([1, D], F32R)
    nc.sync.dma_start(bo_sb, bo_in)
    bg_sb = const.tile([1, DI], F32)
    nc.sync.dma_start(bg_sb, bg_in)
    gamma_row = const.tile([1, D], F32)
    nc.sync.dma_start(gamma_row, gamma_in)
    beta_row = const.tile([1, D], F32)
    nc.sync.dma_start(beta_row, beta_in)
    eps_sb = const.tile([128, 1], F32)
    nc.vector.memset(eps_sb, 1e-5)
    from concourse.masks import make_identity
    ident32 = const.tile([128, 128], F32)
    make_identity(nc, ident32)

    # gamma/beta as columns [128, 2] via PE transpose of [1,128] slices
    def row_to_cols(row, width, out_dtype=F32):
        ntile = width // 128
        p = pst([128, ntile], F32, name="rtc")
        for t in range(ntile):
            nc.tensor.transpose(p[:, t:t + 1], row[0:1, t * 128:(t + 1) * 128],
                                ident32[0:1, 0:1])
        col = const.tile([128, ntile], out_dtype, name=f"col_{row.tensor.name}")
        nc.vector.tensor_copy(col, p)
        return col

    gamma_col = row_to_cols(gamma_row, D)          # [128, 2] f32
    beta_col = row_to_cols(beta_row, D)            # [128, 2] f32 (fp32 matmul lhsT)

    # folded weights: W*_g = gamma (x) W  (q also * SCALE)
    wq_g = const.tile([128, 2, DI], F32R)
    wk_g = const.tile([128, 2, DI], F32R)
    wv_g = const.tile([128, 2, DI], F32R)
    wg_g = const.tile([128, 2, DI], F32R)
    for kt in range(2):
        g = gamma_col[:, kt:kt + 1]
        nc.vector.tensor_scalar(wq_g[:, kt], wq_sb[:, kt], g, SCALE, ALU.mult, ALU.mult)
        nc.vector.tensor_scalar(wk_g[:, kt], wk_sb[:, kt], g, None, ALU.mult)
        nc.vector.tensor_scalar(wv_g[:, kt], wv_sb[:, kt], g, None, ALU.mult)
        nc.vector.tensor_scalar(wg_g[:, kt], wg_sb[:, kt], g, None, ALU.mult)

    # beta @ W rows (raw W) -> per-f bias vectors
    def beta_w_row(w_raw, name, post=None):
        p = pst([1, DI], F32, name=f"bw_{name}")
        for kt in range(2):
            nc.tensor.matmul(p, beta_col[:, kt:kt + 1], w_raw[:, kt],
                             start=(kt == 0), stop=(kt == 1))
        row = const.tile([1, DI], F32R if name == "v" else F32,
                         name=f"bwrow_{name}")
        if post is None:
            nc.vector.tensor_copy(row, p)
        else:
            post(row, p)
        return row

    bwq_row = beta_w_row(wq_sb, "q",
                         post=lambda o, i: nc.vector.tensor_scalar_mul(o, i, SCALE))
    bwk_row = beta_w_row(wk_sb, "k")
    bwv_row = beta_w_row(wv_sb, "v")
    bwg_row = beta_w_row(wg_sb, "g",
                         post=lambda o, i: nc.vector.tensor_tensor(o, i, bg_sb, ALU.add))

    bwq_col = row_to_cols(bwq_row, DI)             # [128, 4] f32
    bwk_col = row_to_cols(bwk_row, DI)
    bwg_col = row_to_cols(bwg_row, DI)

    # ---- bias phase: biasT[h, t'=(j,i)] = Web.T @ edgesT ----
    biasT_dram = dram.tile([HEADS, T_EDGE], F32)
    n_chunks = T_EDGE // EDGE_CHUNK
    for c in range(n_chunks):
        e_sb = edg.tile([128, EDGE_CHUNK], F32R, tag="edg", name="e_sb")
        nc.sync.dma_start(e_sb, edgesT[:, c * EDGE_CHUNK:(c + 1) * EDGE_CHUNK])
        for q in range(EDGE_CHUNK // 512):
            pb = pst([16, 512], F32, name="pb")
            nc.tensor.matmul(pb, web_sb, e_sb[:, q * 512:(q + 1) * 512],
                             start=True, stop=True)
            pb_sb = edg.tile([HEADS, 512], F32, tag="pb_sb", name="pb_sb")
            nc.vector.tensor_copy(pb_sb, pb[0:HEADS])
            nc.sync.dma_start(
                biasT_dram[:, c * EDGE_CHUNK + q * 512:c * EDGE_CHUNK + (q + 1) * 512],
                pb_sb)

    biasT_sb = const.tile([128, 2 * HEADS, N], F32R)   # [j, (h,jt), i]
    for h in range(HEADS):
        for jt in range(2):
            nc.gpsimd.dma_start(
                biasT_sb[:, h * 2 + jt],
                biasT_dram[h, (jt * 128) * N:(jt * 128 + 128) * N]
                .rearrange("(p i) -> p i", p=128))

    # ---- per-row pipeline ----
    for r in range(S_PER_CORE):
        x_sb = work.tile([128, 2, D], F32, tag="x", name="x_sb")
        nc.sync.dma_start(x_sb, x_in[r * N:(r + 1) * N]
                          .rearrange("(t p) d -> p t d", p=128))

        # LayerNorm stats + apply (tokens on partitions)
        xc_sb = work.tile([128, 2, D], F32, tag="xc", name="xc_sb")
        for tt in range(2):
            st = small.tile([128, 6], F32, tag="st", name="st")
            nc.vector.bn_stats(st, x_sb[:, tt])
            mv = small.tile([128, 2], F32, tag="mv", name="mv")
            nc.vector.bn_aggr(mv, st)
            rstd = small.tile([128, 1], F32, tag="rstd", name="rstd")
            nc.scalar.activation(rstd, mv[:, 1:2], AF.Sqrt, bias=eps_sb)
            nc.vector.reciprocal(rstd, rstd)
            nmr = small.tile([128, 1], F32, tag="nmr", name="nmr")
            nc.vector.tensor_mul(nmr, mv[:, 0:1], rstd)
            nc.vector.tensor_scalar_mul(nmr, nmr, -1.0)
            nc.scalar.activation(xc_sb[:, tt], x_sb[:, tt], AF.Identity,
                                 bias=nmr, scale=rstd)

        # x_c^T via PE transpose: [d, tok]
        pxt = pst([128, 512], F32, name="pxt")
        for dt in range(2):
            for tt in range(2):
                nc.tensor.transpose(pxt[:, (dt * 2 + tt) * 128:(dt * 2 + tt + 1) * 128],
                                    xc_sb[:, tt, dt * 128:(dt + 1) * 128], ident32)
        xcT = work.tile([128, 2, N], F32R, tag="xcT", name="xcT")
        for dt in range(2):
            nc.vector.tensor_copy(xcT[:, dt], pxt[:, dt * 256:(dt + 1) * 256])

        # projections
        qT = work.tile([128, 4, N], F32R, tag="qT", name="qT")
        kT = work.tile([128, 4, N], F32R, tag="kT", name="kT")
        gT = work.tile([128, 4, N], F32R, tag="gT", name="gT")
        for w_g, dst, bcol, is_gate in ((wq_g, qT, bwq_col, False),
                                        (wk_g, kT, bwk_col, False),
                                        (wg_g, gT, bwg_col, True)):
            for fp in range(2):
                p = pst([128, 512], name="p_proj")
                for sub in range(2):
                    ft = fp * 2 + sub
                    for kt in range(2):
                        nc.tensor.matmul(p[:, sub * 256:(sub + 1) * 256],
                                         w_g[:, kt, ft * 128:(ft + 1) * 128],
                                         xcT[:, kt],
                                         start=(kt == 0), stop=(kt == 1))
                for sub in range(2):
                    ft = fp * 2 + sub
                    src = p[:, sub * 256:(sub + 1) * 256]
                    if is_gate:
                        nc.scalar.activation(dst[:, ft], src, AF.Sigmoid,
                                             bias=bcol[:, ft:ft + 1])
                    else:
                        nc.vector.tensor_scalar_add(dst[:, ft], src,
                                                    bcol[:, ft:ft + 1])

        # v in natural layout [tok, (h, dh|ones)]
        v_sb = work.tile([128, 2, HEADS, DH + 1], F32R, tag="v", name="v_sb")
        nc.vector.tensor_copy(
            v_sb[:, :, :, DH:DH + 1],
            consts_sb[:, 256:272].rearrange("p (a h c) -> p a h c", a=2, h=HEADS))
        for tt in range(2):
            pv = pst([128, 512], name="pv")
            nc.tensor.matmul(pv, ones_1x128, bwv_row, start=True, stop=False)
            for kt in range(2):
                nc.tensor.matmul(pv, xcT[:, kt, tt * 128:(tt + 1) * 128],
                                 wv_g[:, kt], start=False, stop=(kt == 1))
            nc.vector.tensor_copy(
                v_sb[:, tt, :, 0:DH],
                pv.rearrange("p (h dh) -> p h dh", h=HEADS))

        # attention, head pairs
        gatedT = work.tile([128, 4, N], F32R, tag="gatedT", name="gatedT")
        for pair in range(HEADS // 2):
            pTs = []
            for h in (2 * pair, 2 * pair + 1):
                ph, ft = (h % 2) * 64, h // 2
                s_ps = pst([128, 512], name="s_ps")
                for jt in range(2):
                    nc.tensor.matmul(s_ps[:, jt * 256:(jt + 1) * 256], ident,
                                     biasT_sb[:, h * 2 + jt],
                                     start=True, stop=False)
                    nc.tensor.matmul(s_ps[:, jt * 256:(jt + 1) * 256],
                                     kT[ph:ph + 64, ft, jt * 128:(jt + 1) * 128],
                                     qT[ph:ph + 64, ft],
                                     start=False, stop=True)
                pT = work.tile([128, 512], F32R, tag="pT", bufs=4, name="pT")
                nc.scalar.activation(pT, s_ps, AF.Exp)
                pTs.append(pT)

            av = pst([128, 512], name="av")
            for idx, h in enumerate((2 * pair, 2 * pair + 1)):
                for jt in range(2):
                    nc.tensor.matmul(av[0:DH + 1, idx * 256:(idx + 1) * 256],
                                     v_sb[:, jt, h], pTs[idx][:, jt * 256:(jt + 1) * 256],
                                     start=(jt == 0), stop=(jt == 1))
            rc = small.tile([1, 512], F32R, tag="rc", name="rc")
            nc.vector.reciprocal(rc, av[DH:DH + 1, :])
            bc = pst([64, 512], name="bc")
            nc.tensor.matmul(bc, ones_1x64, rc, start=True, stop=True)
            # bcg = (1/Z broadcast) * sigmoid-gate, then gatedT = av * bcg --
            # each DVE op reads at most one PSUM operand.
            bcg = work.tile([64, 512], F32, tag="bcg", bufs=3, name="bcg")
            for idx, h in enumerate((2 * pair, 2 * pair + 1)):
                ph, ft = (h % 2) * 64, h // 2
                sl = slice(idx * 256, (idx + 1) * 256)
                nc.vector.tensor_tensor(bcg[:, sl], bc[:, sl],
                                        gT[ph:ph + 64, ft], ALU.mult)
                nc.vector.tensor_tensor(gatedT[ph:ph + 64, ft],
                                        av[0:DH, sl], bcg[:, sl], ALU.mult)

        # output projection
        pf = pst([128, 512], name="pf")
        for tt in range(2):
            nc.tensor.matmul(pf[:, tt * 256:(tt + 1) * 256], ones_1x128, bo_sb,
                             start=True, stop=False)
            for kt in range(4):
                nc.tensor.matmul(pf[:, tt * 256:(tt + 1) * 256],
                                 gatedT[:, kt, tt * 128:(tt + 1) * 128],
                                 wo_sb[:, kt], start=False, stop=(kt == 3))
        fout = work.tile([128, 512], F32, tag="fout", name="fout")
        nc.vector.tensor_copy(fout, pf)
        nc.sync.dma_start(out_d[r * N:(r + 1) * N].rearrange("(t p) d -> p t d", p=128),
                          fout.rearrange("p (t d) -> p t d", t=2))

    ctx.close()


_NC_CACHE = {}


def _get_nc():
    if "nc" not in _NC_CACHE:
        _NC_CACHE["nc"] = build_nc()
    return _NC_CACHE["nc"]


def make_in_maps(x, edges, mask, gamma, beta, Wq, Wkv, Wo, bo, Wg, bg, Web):
    f32 = np.float32
    edgesT = np.ascontiguousarray(
        edges[0].transpose(1, 0, 2).reshape(T_EDGE, DE).T.astype(f32))
    consts = np.concatenate(
        [np.eye(128, dtype=f32), np.ones((128, 160), f32)], axis=1)
    shared = {
        "edgesT": edgesT,
        "Wq": np.ascontiguousarray(Wq, f32),
        "Wkv": np.ascontiguousarray(Wkv, f32),
        "Wg": np.ascontiguousarray(Wg, f32),
        "Wo": np.ascontiguousarray(Wo, f32),
        "Web": np.ascontiguousarray(
            np.concatenate([np.asarray(Web, f32),
                            np.zeros((DE, 16 - HEADS), f32)], axis=1)),
        "gamma": np.asarray(gamma, f32).reshape(1, D),
        "beta": np.asarray(beta, f32).reshape(1, D),
        "bo": np.asarray(bo, f32).reshape(1, D),
        "bg": np.asarray(bg, f32).reshape(1, DI),
        "consts": consts,
    }
    x0 = np.asarray(x, f32)[0]   # [S, N, D]
    in_maps = []
    for c in range(N_CORES):
        xs = np.ascontiguousarray(
            x0[c * S_PER_CORE:(c + 1) * S_PER_CORE].reshape(S_PER_CORE * N, D))
        in_maps.append({"x": xs, **shared})
    return in_maps


def kernel(x, edges, mask, gamma, beta, Wq, Wkv, Wo, bo, Wg, bg, Web,
           **run_kwargs):
    nc = _get_nc()
    in_maps = make_in_maps(x, edges, mask, gamma, beta, Wq, Wkv, Wo, bo, Wg, bg, Web)
    res = run_bass_kernel_spmd(nc, in_maps, core_ids=list(range(N_CORES)),
                               **run_kwargs)
    outs = [res.results[c]["out"].reshape(S_PER_CORE, N, D) for c in range(N_CORES)]
    full = np.concatenate(outs, axis=0)[None]   # [1, S, N, D]
    if run_kwargs:
        kernel.last_results = res
    return full


# revision 12
# speedup vs baseline: 1.6798x; 1.6798x over previous
"""AxialAttention (MSA row attention) Trainium2 Bass kernel, 8-core SPMD.

Sharding: the s=128 MSA-row axis is split 16 rows/core across 8 cores.
Params are replicated; the pairwise attention bias is recomputed on every
core from a CPU-pre-transposed (and bf16-cast) copy of `edges`.

Per-core dataflow (matmul operands in bf16, accumulation in fp32 PSUM):
  LayerNorm (tokens on partitions, bn_stats)  ->  PE-transpose x_c
  qT/kT/gT = W_g.T @ x_cT   (f on partitions)     v in natural layout
  scoresT[j,i] = bias^T (identity-injected into PSUM) + kT.T@qT
  P^T = exp(scoresT)        (no max subtraction: logits bounded ~+-2)
  Z^T via N=1 matmuls (i on partitions) -> wide DVE reciprocal ->
  PE-transpose back to a row -> K=1 ones matmul broadcasts 1/Z;
  gatedT = (attn @ v) * sigmoid-gate * 1/Z on DVE
  out = gatedT.T @ Wo + bo (K=1 inject) -> SBUF -> DRAM
The bias phase (Web.T @ edgesT) is interleaved with the first rows'
projections; attention trails projections by ATT_LAG rows so the row
pipeline overlaps the bias DMA.
"""
import sys

if "/opt/trn_rl_repo" not in sys.path:
    sys.path.insert(0, "/opt/trn_rl_repo")

import numpy as np
import ml_dtypes

import concourse.bass as bass
import concourse.tile as tile
from concourse import bacc, mybir
from concourse.bass_utils import run_bass_kernel_spmd

F32 = mybir.dt.float32
BF16 = mybir.dt.bfloat16
AF = mybir.ActivationFunctionType
ALU = mybir.AluOpType

N_CORES = 8
S = 128                 # MSA rows (axial batch)
S_PER_CORE = S // N_CORES
N = 256                 # sequence positions per row
D = 256                 # node dim
HEADS = 8
DH = 64                 # head dim
DI = HEADS * DH         # 512
DE = 128                # edge dim
T_EDGE = N * N          # 65536 flattened (j,i) pairs
EDGE_CHUNK = 2048       # t' per bias-phase chunk (bf16: 0.5 MB)
N_CHUNKS = T_EDGE // EDGE_CHUNK
SCALE = DH ** -0.5
ATT_LAG = 5             # attention trails projections by this many rows


def build_nc():
    nc = bacc.Bacc("TRN2", target_bir_lowering=False, debug=False,
                   num_devices=N_CORES)

    io = {}
    io["x"] = nc.dram_tensor("x", [S_PER_CORE * N, D], F32, kind="ExternalInput").ap()
    io["edgesT"] = nc.dram_tensor("edgesT", [DE, T_EDGE], BF16, kind="ExternalInput").ap()
    io["Wq"] = nc.dram_tensor("Wq", [D, DI], F32, kind="ExternalInput").ap()
    io["Wkv"] = nc.dram_tensor("Wkv", [D, 2 * DI], F32, kind="ExternalInput").ap()
    io["Wg"] = nc.dram_tensor("Wg", [D, DI], F32, kind="ExternalInput").ap()
    io["Wo"] = nc.dram_tensor("Wo", [DI, D], F32, kind="ExternalInput").ap()
    io["Web"] = nc.dram_tensor("Web", [DE, 64], BF16, kind="ExternalInput").ap()
    io["gamma"] = nc.dram_tensor("gamma", [1, D], F32, kind="ExternalInput").ap()
    io["beta"] = nc.dram_tensor("beta", [1, D], F32, kind="ExternalInput").ap()
    io["bo"] = nc.dram_tensor("bo", [1, D], BF16, kind="ExternalInput").ap()
    io["bg"] = nc.dram_tensor("bg", [1, DI], F32, kind="ExternalInput").ap()
    io["consts"] = nc.dram_tensor("consts", [128, 288], BF16, kind="ExternalInput").ap()
    io["out"] = nc.dram_tensor("out", [S_PER_CORE * N, D], F32, kind="ExternalOutput").ap()

    with tile.TileContext(nc) as tc, nc.allow_low_precision(
        reason="bf16 matmul operands; fp32 PSUM accumulation"
    ):
        _emit(nc, tc, io)
    nc.compile()
    return nc


def _emit(nc, tc, io):
    from contextlib import ExitStack
    from concourse.masks import make_identity
    ctx = ExitStack()
    const = ctx.enter_context(tc.tile_pool(name="const", bufs=1))
    work = ctx.enter_context(tc.tile_pool(name="work", bufs=2))
    small = ctx.enter_context(tc.tile_pool(name="small", bufs=6))
    edg = ctx.enter_context(tc.tile_pool(name="edg", bufs=3))
    ps = ctx.enter_context(tc.tile_pool(name="ps", bufs=8, space="PSUM"))
    dram = ctx.enter_context(tc.tile_pool(name="dram", bufs=1, space="DRAM"))

    def pst(shape, dtype=F32, name="pst"):
        return ps.tile(shape, dtype, tag="ps", name=name)

    RB = ATT_LAG + 2        # buffering for tiles that live proj -> attention

    # ---- constants / weights ----
    consts_sb = const.tile([128, 288], BF16)
    nc.sync.dma_start(consts_sb, io["consts"])
    ident_bf = consts_sb[:, 0:128]
    ones_1x128 = consts_sb[0:1, 128:256]
    ones_1x64 = consts_sb[0:1, 128:192]
    ones_col = consts_sb[:, 128:129]          # [128, 1] ones

    wq_sb = const.tile([128, 2, DI], F32)
    nc.sync.dma_start(wq_sb, io["Wq"].rearrange("(kt p) f -> p kt f", p=128))
    wk_sb = const.tile([128, 2, DI], F32)
    nc.sync.dma_start(wk_sb, io["Wkv"][:, 0:DI].rearrange("(kt p) f -> p kt f", p=128))
    wv_sb = const.tile([128, 2, DI], F32)
    nc.sync.dma_start(wv_sb, io["Wkv"][:, DI:2 * DI].rearrange("(kt p) f -> p kt f", p=128))
    wg_sb = const.tile([128, 2, DI], F32)
    nc.sync.dma_start(wg_sb, io["Wg"].rearrange("(kt p) f -> p kt f", p=128))
    wo_sb = const.tile([128, 4, D], BF16)
    nc.gpsimd.dma_start(wo_sb, io["Wo"].rearrange("(kt p) f -> p kt f", p=128))
    web_sb = const.tile([128, 64], BF16)
    nc.sync.dma_start(web_sb, io["Web"])
    bo_sb = const.tile([1, D], BF16)
    nc.sync.dma_start(bo_sb, io["bo"])
    bg_sb = const.tile([1, DI], F32)
    nc.sync.dma_start(bg_sb, io["bg"])
    gamma_row = const.tile([1, D], F32)
    nc.sync.dma_start(gamma_row, io["gamma"])
    beta_row = const.tile([1, D], F32)
    nc.sync.dma_start(beta_row, io["beta"])
    eps_sb = const.tile([128, 1], F32)
    nc.vector.memset(eps_sb, 1e-5)
    ident32 = const.tile([128, 128], F32)
    make_identity(nc, ident32)

    # gamma/beta as per-partition columns via PE transpose of [1,128] slices
    def row_to_cols(row, width):
        ntile = width // 128
        p = pst([128, ntile], F32, name="rtc")
        for t in range(ntile):
            nc.tensor.transpose(p[:, t:t + 1], row[0:1, t * 128:(t + 1) * 128],
                                ident32[0:1, 0:1])
        col = const.tile([128, ntile], F32, name=f"col_{row.tensor.name}")
        nc.vector.tensor_copy(col, p)
        return col

    gamma_col = row_to_cols(gamma_row, D)
    beta_col = row_to_cols(beta_row, D)

    # folded weights (bf16): W*_g = gamma (x) W  (q also * SCALE)
    wq_g = const.tile([128, 2, DI], BF16)
    wk_g = const.tile([128, 2, DI], BF16)
    wv_g = const.tile([128, 2, DI], BF16)
    wg_g = const.tile([128, 2, DI], BF16)
    for kt in range(2):
        g = gamma_col[:, kt:kt + 1]
        nc.vector.tensor_scalar(wq_g[:, kt], wq_sb[:, kt], g, SCALE, ALU.mult, ALU.mult)
        nc.vector.tensor_scalar(wk_g[:, kt], wk_sb[:, kt], g, None, ALU.mult)
        nc.vector.tensor_scalar(wv_g[:, kt], wv_sb[:, kt], g, None, ALU.mult)
        nc.vector.tensor_scalar(wg_g[:, kt], wg_sb[:, kt], g, None, ALU.mult)

    # beta @ W rows (raw fp32 W, fp32 matmul) -> per-f bias vectors
    def beta_w_row(w_raw, name, dtype, post=None):
        p = pst([1, DI], F32, name=f"bw_{name}")
        for kt in range(2):
            nc.tensor.matmul(p, beta_col[:, kt:kt + 1], w_raw[:, kt],
                             start=(kt == 0), stop=(kt == 1))
        row = const.tile([1, DI], dtype, name=f"bwrow_{name}")
        if post is None:
            nc.vector.tensor_copy(row, p)
        else:
            post(row, p)
        return row

    bwq_row = beta_w_row(wq_sb, "q", F32,
                         post=lambda o, i: nc.vector.tensor_scalar_mul(o, i, SCALE))
    bwk_row = beta_w_row(wk_sb, "k", F32)
    bwv_row = beta_w_row(wv_sb, "v", BF16)
    bwg_row = beta_w_row(wg_sb, "g", F32,
                         post=lambda o, i: nc.vector.tensor_tensor(o, i, bg_sb, ALU.add))

    bwq_col = row_to_cols(bwq_row, DI)             # [128, 4] f32
    bwk_col = row_to_cols(bwk_row, DI)
    bwg_col = row_to_cols(bwg_row, DI)

    # ---- bias phase (emitted interleaved below) ----
    biasT_dram = dram.tile([HEADS, T_EDGE], BF16)
    biasT_sb = const.tile([128, 2 * HEADS, N], BF16)   # [j, (h,jt), i]

    def emit_bias_chunk(c):
        e_sb = edg.tile([128, EDGE_CHUNK], BF16, tag="edg", name="e_sb")
        nc.sync.dma_start(e_sb, io["edgesT"][:, c * EDGE_CHUNK:(c + 1) * EDGE_CHUNK])
        for half in range(2):
            pb = pst([128, 512], F32, name="pb")
            for sub in range(2):
                q = half * 2 + sub
                nc.tensor.matmul(pb[sub * 64:(sub + 1) * 64],
                                 web_sb, e_sb[:, q * 512:(q + 1) * 512],
                                 start=True, stop=True)
            pb_sb = edg.tile([128, 512], BF16, tag="pb_sb", name="pb_sb")
            if half == 0:
                nc.vector.tensor_copy(pb_sb, pb)
            else:
                nc.scalar.copy(pb_sb, pb)
            for sub in range(2):
                q = half * 2 + sub
                off = c * EDGE_CHUNK + q * 512
                nc.sync.dma_start(biasT_dram[:, off:off + 512],
                                  pb_sb[sub * 64:sub * 64 + HEADS])

    def emit_bias_backs():
        for h in range(HEADS):
            for jt in range(2):
                nc.sync.dma_start(
                    biasT_sb[:, h * 2 + jt],
                    biasT_dram[h, (jt * 128) * N:(jt * 128 + 128) * N]
                    .rearrange("(p i) -> p i", p=128))

    # ---- per-row: LayerNorm + projections ----
    row_tiles = {}

    def emit_proj(r):
        x_sb = work.tile([128, 2, D], F32, tag="x", bufs=3, name="x_sb")
        nc.sync.dma_start(x_sb, io["x"][r * N:(r + 1) * N]
                          .rearrange("(t p) d -> p t d", p=128))

        xc_sb = work.tile([128, 2, D], BF16, tag="xc", bufs=3, name="xc_sb")
        for tt in range(2):
            st = small.tile([128, 6], F32, tag="st", name="st")
            nc.vector.bn_stats(st, x_sb[:, tt])
            mv = small.tile([128, 2], F32, tag="mv", name="mv")
            nc.vector.bn_aggr(mv, st)
            rstd = small.tile([128, 1], F32, tag="rstd", name="rstd")
            nc.scalar.activation(rstd, mv[:, 1:2], AF.Sqrt, bias=eps_sb)
            nc.vector.reciprocal(rstd, rstd)
            nmr = small.tile([128, 1], F32, tag="nmr", name="nmr")
            nc.vector.tensor_mul(nmr, mv[:, 0:1], rstd)
            nc.vector.tensor_scalar_mul(nmr, nmr, -1.0)
            nc.scalar.activation(xc_sb[:, tt], x_sb[:, tt], AF.Identity,
                                 bias=nmr, scale=rstd)

        pxt = pst([128, 512], BF16, name="pxt")
        for dt in range(2):
            for tt in range(2):
                nc.tensor.transpose(pxt[:, (dt * 2 + tt) * 128:(dt * 2 + tt + 1) * 128],
                                    xc_sb[:, tt, dt * 128:(dt + 1) * 128], ident_bf)
        xcT = work.tile([128, 2, N], BF16, tag="xcT", bufs=3, name="xcT")
        for dt in range(2):
            nc.vector.tensor_copy(xcT[:, dt], pxt[:, dt * 256:(dt + 1) * 256])

        qT = work.tile([128, 4, N], BF16, tag="qT", bufs=RB, name="qT")
        kT = work.tile([128, 4, N], BF16, tag="kT", bufs=RB, name="kT")
        gT = work.tile([128, 4, N], BF16, tag="gT", bufs=RB, name="gT")
        for w_g, dst, bcol, is_gate in ((wq_g, qT, bwq_col, False),
                                        (wk_g, kT, bwk_col, False),
                                        (wg_g, gT, bwg_col, True)):
            for fp in range(2):
                p = pst([128, 512], name="p_proj")
                for sub in range(2):
                    ft = fp * 2 + sub
                    for kt in range(2):
                        nc.tensor.matmul(p[:, sub * 256:(sub + 1) * 256],
                                         w_g[:, kt, ft * 128:(ft + 1) * 128],
                                         xcT[:, kt],
                                         start=(kt == 0), stop=(kt == 1))
                for sub in range(2):
                    ft = fp * 2 + sub
                    src = p[:, sub * 256:(sub + 1) * 256]
                    if is_gate:
                        nc.scalar.activation(dst[:, ft], src, AF.Sigmoid,
                                             bias=bcol[:, ft:ft + 1])
                    else:
                        nc.vector.tensor_scalar_add(dst[:, ft], src,
                                                    bcol[:, ft:ft + 1])

        v_sb = work.tile([128, 2, DI], BF16, tag="v", bufs=RB, name="v_sb")
        for tt in range(2):
            pv = pst([128, 512], name="pv")
            nc.tensor.matmul(pv, ones_1x128, bwv_row, start=True, stop=False)
            for kt in range(2):
                nc.tensor.matmul(pv, xcT[:, kt, tt * 128:(tt + 1) * 128],
                                 wv_g[:, kt], start=False, stop=(kt == 1))
            nc.vector.tensor_copy(v_sb[:, tt], pv)

        row_tiles[r] = (qT, kT, gT, v_sb)

    # ---- per-row: attention + output projection ----
    def emit_attn(r):
        qT, kT, gT, v_sb = row_tiles.pop(r)
        gatedT = work.tile([128, 4, N], BF16, tag="gatedT", bufs=3, name="gatedT")
        for pair in range(HEADS // 2):
            h0 = 2 * pair
            pTs = []
            for h in (h0, h0 + 1):
                ph, ft = (h % 2) * 64, h // 2
                s_ps = pst([128, 512], name="s_ps")
                for jt in range(2):
                    nc.tensor.matmul(s_ps[:, jt * 256:(jt + 1) * 256], ident_bf,
                                     biasT_sb[:, h * 2 + jt],
                                     start=True, stop=False)
                    nc.tensor.matmul(s_ps[:, jt * 256:(jt + 1) * 256],
                                     kT[ph:ph + 64, ft, jt * 128:(jt + 1) * 128],
                                     qT[ph:ph + 64, ft],
                                     start=False, stop=True)
                pT = work.tile([128, 512], BF16, tag="pT", bufs=2 * RB, name="pT")
                nc.scalar.activation(pT, s_ps, AF.Exp)
                pTs.append(pT)

            # Z^T: [i on partitions] via N=1 matmuls over the j tiles
            zt = pst([128, 4], F32, name="zt")
            for idx in range(2):
                for it in range(2):
                    for jt in range(2):
                        nc.tensor.matmul(
                            zt[:, idx * 2 + it:idx * 2 + it + 1],
                            pTs[idx][:, jt * 256 + it * 128:jt * 256 + (it + 1) * 128],
                            ones_col, start=(jt == 0), stop=(jt == 1))
            recipT = small.tile([128, 4], BF16, tag="recipT", name="recipT")
            nc.vector.reciprocal(recipT, zt)

            av = pst([64, 512], name="av")
            for idx, h in enumerate((h0, h0 + 1)):
                for jt in range(2):
                    nc.tensor.matmul(av[:, idx * 256:(idx + 1) * 256],
                                     v_sb[:, jt, h * DH:(h + 1) * DH],
                                     pTs[idx][:, jt * 256:(jt + 1) * 256],
                                     start=(jt == 0), stop=(jt == 1))
            # 1/Z broadcast: replicate each recipT column 64-wide on DVE, then
            # PE-transpose the block so every output row is 1/Z for that i range
            rwide = small.tile([128, 4, 64], BF16, tag="rwide", name="rwide")
            nc.vector.tensor_copy(rwide, recipT[:, :, None].to_broadcast([128, 4, 64]))
            bc = pst([64, 512], BF16, name="bc")
            for c in range(4):
                nc.tensor.transpose(bc[:, c * 128:(c + 1) * 128],
                                    rwide[:, c], ident_bf)
            bcg = work.tile([64, 512], F32, tag="bcg", bufs=3, name="bcg")
            for idx, h in enumerate((h0, h0 + 1)):
                ph, ft = (h % 2) * 64, h // 2
                sl = slice(idx * 256, (idx + 1) * 256)
                nc.vector.tensor_tensor(bcg[:, sl], bc[:, sl],
                                        gT[ph:ph + 64, ft], ALU.mult)
                nc.vector.tensor_tensor(gatedT[ph:ph + 64, ft],
                                        av[:, sl], bcg[:, sl], ALU.mult)

        pf = pst([128, 512], name="pf")
        for tt in range(2):
            nc.tensor.matmul(pf[:, tt * 256:(tt + 1) * 256], ones_1x128, bo_sb,
                             start=True, stop=False)
            for kt in range(4):
                nc.tensor.matmul(pf[:, tt * 256:(tt + 1) * 256],
                                 gatedT[:, kt, tt * 128:(tt + 1) * 128],
                                 wo_sb[:, kt], start=False, stop=(kt == 3))
        fout = work.tile([128, 512], F32, tag="fout", bufs=3, name="fout")
        nc.vector.tensor_copy(fout, pf)
        nc.sync.dma_start(io["out"][r * N:(r + 1) * N].rearrange("(t p) d -> p t d", p=128),
                          fout.rearrange("p (t d) -> p t d", t=2))

    # ---- interleaved emission ----
    for r in range(S_PER_CORE + ATT_LAG):
        if r < S_PER_CORE:
            if r < 4:
                for c in range(8 * r, 8 * r + 8):
                    emit_bias_chunk(c)
            emit_proj(r)
            if r == 3:
                emit_bias_backs()
        if r >= ATT_LAG:
            emit_attn(r - ATT_LAG)

    ctx.close()


_NC_CACHE = {}


def _get_nc():
    if "nc" not in _NC_CACHE:
        _NC_CACHE["nc"] = build_nc()
    return _NC_CACHE["nc"]


def make_in_maps(x, edges, mask, gamma, beta, Wq, Wkv, Wo, bo, Wg, bg, Web):
    f32 = np.float32
    bf16 = ml_dtypes.bfloat16
    edgesT = np.ascontiguousarray(
        edges[0].transpose(1, 0, 2).reshape(T_EDGE, DE).T).astype(bf16)
    consts = np.concatenate(
        [np.eye(128, dtype=f32), np.ones((128, 160), f32)], axis=1).astype(bf16)
    shared = {
        "edgesT": edgesT,
        "Wq": np.ascontiguousarray(Wq, f32),
        "Wkv": np.ascontiguousarray(Wkv, f32),
        "Wg": np.ascontiguousarray(Wg, f32),
        "Wo": np.ascontiguousarray(Wo, f32),
        "Web": np.concatenate([np.asarray(Web, f32),
                               np.zeros((DE, 64 - HEADS), f32)], axis=1).astype(bf16),
        "gamma": np.asarray(gamma, f32).reshape(1, D),
        "beta": np.asarray(beta, f32).reshape(1, D),
        "bo": np.asarray(bo, f32).reshape(1, D).astype(bf16),
        "bg": np.asarray(bg, f32).reshape(1, DI),
        "consts": consts,
    }
    x0 = np.asarray(x, f32)[0]   # [S, N, D]
    in_maps = []
    for c in range(N_CORES):
        xs = np.ascontiguousarray(
            x0[c * S_PER_CORE:(c + 1) * S_PER_CORE].reshape(S_PER_CORE * N, D))
        in_maps.append({"x": xs, **shared})
    return in_maps


def kernel(x, edges, mask, gamma, beta, Wq, Wkv, Wo, bo, Wg, bg, Web,
           **run_kwargs):
    nc = _get_nc()
    in_maps = make_in_maps(x, edges, mask, gamma, beta, Wq, Wkv, Wo, bo, Wg, bg, Web)
    res = run_bass_kernel_spmd(nc, in_maps, core_ids=list(range(N_CORES)),
                               **run_kwargs)
    outs = [res.results[c]["out"].reshape(S_PER_CORE, N, D) for c in range(N_CORES)]
    full = np.concatenate(outs, axis=0)[None]   # [1, S, N, D]
    if run_kwargs:
        kernel.last_results = res
    return full


# revision 14
# speedup vs baseline: 1.7801x; 1.0597x over previous
"""AxialAttention (MSA row attention) Trainium2 Bass kernel, 8-core SPMD.

Sharding: the s=128 MSA-row axis is split 16 rows/core across 8 cores.
Params are replicated; the pairwise attention bias is recomputed on every
core from a CPU-pre-transposed (and bf16-cast) copy of `edges`.

Per-core dataflow (matmul operands in bf16, accumulation in fp32 PSUM):
  LayerNorm (tokens on partitions, bn_stats)  ->  PE-transpose x_c
  qT/kT/gT = W_g.T @ x_cT   (f on partitions)     v in natural layout
  scoresT[j,i] = bias^T (identity-injected into PSUM) + kT.T@qT
  P^T = exp(scoresT)        (no max subtraction: logits bounded ~+-2)
  Z^T via N=1 matmuls (i on partitions) -> wide DVE reciprocal ->
  PE-transpose back to a row -> K=1 ones matmul broadcasts 1/Z;
  gatedT = (attn @ v) * sigmoid-gate * 1/Z on DVE
  out = gatedT.T @ Wo + bo (K=1 inject) -> SBUF -> DRAM
The bias phase (Web.T @ edgesT) is interleaved with the first rows'
projections; attention trails projections by ATT_LAG rows so the row
pipeline overlaps the bias DMA.
"""
import sys

if "/opt/trn_rl_repo" not in sys.path:
    sys.path.insert(0, "/opt/trn_rl_repo")

import numpy as np
import ml_dtypes

import concourse.bass as bass
import concourse.tile as tile
from concourse import bacc, mybir
from concourse.bass_utils import run_bass_kernel_spmd

F32 = mybir.dt.float32
BF16 = mybir.dt.bfloat16
AF = mybir.ActivationFunctionType
ALU = mybir.AluOpType

N_CORES = 8
S = 128                 # MSA rows (axial batch)
S_PER_CORE = S // N_CORES
N = 256                 # sequence positions per row
D = 256                 # node dim
HEADS = 8
DH = 64                 # head dim
DI = HEADS * DH         # 512
DE = 128                # edge dim
T_EDGE = N * N          # 65536 flattened (j,i) pairs
EDGE_CHUNK = 2048       # t' per bias-phase chunk (bf16: 0.5 MB)
N_CHUNKS = T_EDGE // EDGE_CHUNK
SCALE = DH ** -0.5
ATT_LAG = 5             # attention trails projections by this many rows


def build_nc():
    nc = bacc.Bacc("TRN2", target_bir_lowering=False, debug=False,
                   num_devices=N_CORES)

    io = {}
    io["x"] = nc.dram_tensor("x", [S_PER_CORE * N, D], F32, kind="ExternalInput").ap()
    io["edgesT"] = nc.dram_tensor("edgesT", [DE, T_EDGE], BF16, kind="ExternalInput").ap()
    io["Wq"] = nc.dram_tensor("Wq", [D, DI], F32, kind="ExternalInput").ap()
    io["Wkv"] = nc.dram_tensor("Wkv", [D, 2 * DI], F32, kind="ExternalInput").ap()
    io["Wg"] = nc.dram_tensor("Wg", [D, DI], F32, kind="ExternalInput").ap()
    io["Wo"] = nc.dram_tensor("Wo", [DI, D], F32, kind="ExternalInput").ap()
    io["Web"] = nc.dram_tensor("Web", [DE, 64], BF16, kind="ExternalInput").ap()
    io["gamma"] = nc.dram_tensor("gamma", [1, D], F32, kind="ExternalInput").ap()
    io["beta"] = nc.dram_tensor("beta", [1, D], F32, kind="ExternalInput").ap()
    io["bo"] = nc.dram_tensor("bo", [1, D], BF16, kind="ExternalInput").ap()
    io["bg"] = nc.dram_tensor("bg", [1, DI], F32, kind="ExternalInput").ap()
    io["consts"] = nc.dram_tensor("consts", [128, 288], BF16, kind="ExternalInput").ap()
    io["out"] = nc.dram_tensor("out", [S_PER_CORE * N, D], F32, kind="ExternalOutput").ap()

    with tile.TileContext(nc) as tc, nc.allow_low_precision(
        reason="bf16 matmul operands; fp32 PSUM accumulation"
    ):
        _emit(nc, tc, io)
    nc.compile()
    return nc


def _emit(nc, tc, io):
    from contextlib import ExitStack
    from concourse.masks import make_identity
    ctx = ExitStack()
    const = ctx.enter_context(tc.tile_pool(name="const", bufs=1))
    work = ctx.enter_context(tc.tile_pool(name="work", bufs=2))
    small = ctx.enter_context(tc.tile_pool(name="small", bufs=6))
    edg = ctx.enter_context(tc.tile_pool(name="edg", bufs=4))
    ps = ctx.enter_context(tc.tile_pool(name="ps", bufs=8, space="PSUM"))
    dram = ctx.enter_context(tc.tile_pool(name="dram", bufs=1, space="DRAM"))

    def pst(shape, dtype=F32, name="pst"):
        return ps.tile(shape, dtype, tag="ps", name=name)

    RB = ATT_LAG + 2        # buffering for tiles that live proj -> attention

    # ---- constants / weights ----
    consts_sb = const.tile([128, 288], BF16)
    nc.sync.dma_start(consts_sb, io["consts"])
    ident_bf = consts_sb[:, 0:128]
    ones_1x128 = consts_sb[0:1, 128:256]
    ones_1x64 = consts_sb[0:1, 128:192]
    ones_col = consts_sb[:, 128:129]          # [128, 1] ones

    wq_sb = const.tile([128, 2, DI], F32)
    nc.sync.dma_start(wq_sb, io["Wq"].rearrange("(kt p) f -> p kt f", p=128))
    wk_sb = const.tile([128, 2, DI], F32)
    nc.sync.dma_start(wk_sb, io["Wkv"][:, 0:DI].rearrange("(kt p) f -> p kt f", p=128))
    wv_sb = const.tile([128, 2, DI], F32)
    nc.sync.dma_start(wv_sb, io["Wkv"][:, DI:2 * DI].rearrange("(kt p) f -> p kt f", p=128))
    wg_sb = const.tile([128, 2, DI], F32)
    nc.sync.dma_start(wg_sb, io["Wg"].rearrange("(kt p) f -> p kt f", p=128))
    wo_sb = const.tile([128, 4, D], BF16)
    nc.gpsimd.dma_start(wo_sb, io["Wo"].rearrange("(kt p) f -> p kt f", p=128))
    web_sb = const.tile([128, 64], BF16)
    nc.sync.dma_start(web_sb, io["Web"])
    bo_sb = const.tile([1, D], BF16)
    nc.sync.dma_start(bo_sb, io["bo"])
    bg_sb = const.tile([1, DI], F32)
    nc.sync.dma_start(bg_sb, io["bg"])
    gamma_row = const.tile([1, D], F32)
    nc.sync.dma_start(gamma_row, io["gamma"])
    beta_row = const.tile([1, D], F32)
    nc.sync.dma_start(beta_row, io["beta"])
    eps_sb = const.tile([128, 1], F32)
    nc.vector.memset(eps_sb, 1e-5)
    ident32 = const.tile([128, 128], F32)
    make_identity(nc, ident32)

    # gamma/beta as per-partition columns via PE transpose of [1,128] slices
    def row_to_cols(row, width):
        ntile = width // 128
        p = pst([128, ntile], F32, name="rtc")
        for t in range(ntile):
            nc.tensor.transpose(p[:, t:t + 1], row[0:1, t * 128:(t + 1) * 128],
                                ident32[0:1, 0:1])
        col = const.tile([128, ntile], F32, name=f"col_{row.tensor.name}")
        nc.vector.tensor_copy(col, p)
        return col

    gamma_col = row_to_cols(gamma_row, D)
    beta_col = row_to_cols(beta_row, D)

    # folded weights (bf16): W*_g = gamma (x) W  (q also * SCALE)
    wq_g = const.tile([128, 2, DI], BF16)
    wk_g = const.tile([128, 2, DI], BF16)
    wv_g = const.tile([128, 2, DI], BF16)
    wg_g = const.tile([128, 2, DI], BF16)
    for kt in range(2):
        g = gamma_col[:, kt:kt + 1]
        nc.vector.tensor_scalar(wq_g[:, kt], wq_sb[:, kt], g, SCALE, ALU.mult, ALU.mult)
        nc.vector.tensor_scalar(wk_g[:, kt], wk_sb[:, kt], g, None, ALU.mult)
        nc.vector.tensor_scalar(wv_g[:, kt], wv_sb[:, kt], g, None, ALU.mult)
        nc.vector.tensor_scalar(wg_g[:, kt], wg_sb[:, kt], g, None, ALU.mult)

    # beta @ W rows (raw fp32 W, fp32 matmul) -> per-f bias vectors
    def beta_w_row(w_raw, name, dtype, post=None):
        p = pst([1, DI], F32, name=f"bw_{name}")
        for kt in range(2):
            nc.tensor.matmul(p, beta_col[:, kt:kt + 1], w_raw[:, kt],
                             start=(kt == 0), stop=(kt == 1))
        row = const.tile([1, DI], dtype, name=f"bwrow_{name}")
        if post is None:
            nc.vector.tensor_copy(row, p)
        else:
            post(row, p)
        return row

    bwq_row = beta_w_row(wq_sb, "q", F32,
                         post=lambda o, i: nc.vector.tensor_scalar_mul(o, i, SCALE))
    bwk_row = beta_w_row(wk_sb, "k", F32)
    bwv_row = beta_w_row(wv_sb, "v", BF16)
    bwg_row = beta_w_row(wg_sb, "g", F32,
                         post=lambda o, i: nc.vector.tensor_tensor(o, i, bg_sb, ALU.add))

    bwq_col = row_to_cols(bwq_row, DI)             # [128, 4] f32
    bwk_col = row_to_cols(bwk_row, DI)
    bwg_col = row_to_cols(bwg_row, DI)

    # ---- bias phase (emitted interleaved below) ----
    biasT_dram = dram.tile([HEADS, T_EDGE], BF16)
    biasT_sb = const.tile([128, 2 * HEADS, N], BF16)   # [j, (h,jt), i]

    def emit_bias_chunk(c):
        e_sb = edg.tile([128, EDGE_CHUNK], BF16, tag="edg", name="e_sb")
        nc.sync.dma_start(e_sb, io["edgesT"][:, c * EDGE_CHUNK:(c + 1) * EDGE_CHUNK])
        for half in range(2):
            pb = pst([128, 512], F32, name="pb")
            for sub in range(2):
                q = half * 2 + sub
                nc.tensor.matmul(pb[sub * 64:(sub + 1) * 64],
                                 web_sb, e_sb[:, q * 512:(q + 1) * 512],
                                 start=True, stop=True)
            pb_sb = edg.tile([128, 512], BF16, tag="pb_sb", name="pb_sb")
            nc.scalar.copy(pb_sb, pb)
            for sub in range(2):
                q = half * 2 + sub
                off = c * EDGE_CHUNK + q * 512
                nc.gpsimd.dma_start(biasT_dram[:, off:off + 512],
                                  pb_sb[sub * 64:sub * 64 + HEADS])

    def emit_bias_backs():
        for h in range(HEADS):
            for jt in range(2):
                nc.sync.dma_start(
                    biasT_sb[:, h * 2 + jt],
                    biasT_dram[h, (jt * 128) * N:(jt * 128 + 128) * N]
                    .rearrange("(p i) -> p i", p=128))

    # ---- per-row: LayerNorm + projections ----
    row_tiles = {}

    def emit_proj(r):
        x_sb = work.tile([128, 2, D], F32, tag="x", bufs=3, name="x_sb")
        nc.sync.dma_start(x_sb, io["x"][r * N:(r + 1) * N]
                          .rearrange("(t p) d -> p t d", p=128))

        xc_sb = work.tile([128, 2, D], BF16, tag="xc", bufs=3, name="xc_sb")
        for tt in range(2):
            st = small.tile([128, 6], F32, tag="st", name="st")
            nc.vector.bn_stats(st, x_sb[:, tt])
            mv = small.tile([128, 2], F32, tag="mv", name="mv")
            nc.vector.bn_aggr(mv, st)
            rstd = small.tile([128, 1], F32, tag="rstd", name="rstd")
            nc.scalar.activation(rstd, mv[:, 1:2], AF.Sqrt, bias=eps_sb)
            nc.vector.reciprocal(rstd, rstd)
            nmr = small.tile([128, 1], F32, tag="nmr", name="nmr")
            nc.vector.tensor_mul(nmr, mv[:, 0:1], rstd)
            nc.vector.tensor_scalar_mul(nmr, nmr, -1.0)
            nc.scalar.activation(xc_sb[:, tt], x_sb[:, tt], AF.Identity,
                                 bias=nmr, scale=rstd)

        pxt = pst([128, 512], BF16, name="pxt")
        for dt in range(2):
            for tt in range(2):
                nc.tensor.transpose(pxt[:, (dt * 2 + tt) * 128:(dt * 2 + tt + 1) * 128],
                                    xc_sb[:, tt, dt * 128:(dt + 1) * 128], ident_bf)
        xcT = work.tile([128, 2, N], BF16, tag="xcT", bufs=3, name="xcT")
        for dt in range(2):
            nc.vector.tensor_copy(xcT[:, dt], pxt[:, dt * 256:(dt + 1) * 256])

        qT = work.tile([128, 4, N], BF16, tag="qT", bufs=RB, name="qT")
        kT = work.tile([128, 4, N], BF16, tag="kT", bufs=RB, name="kT")
        gT = work.tile([128, 4, N], BF16, tag="gT", bufs=RB, name="gT")
        for w_g, dst, bcol, is_gate in ((wq_g, qT, bwq_col, False),
                                        (wk_g, kT, bwk_col, False),
                                        (wg_g, gT, bwg_col, True)):
            for fp in range(2):
                p = pst([128, 512], name="p_proj")
                for sub in range(2):
                    ft = fp * 2 + sub
                    for kt in range(2):
                        nc.tensor.matmul(p[:, sub * 256:(sub + 1) * 256],
                                         w_g[:, kt, ft * 128:(ft + 1) * 128],
                                         xcT[:, kt],
                                         start=(kt == 0), stop=(kt == 1))
                for sub in range(2):
                    ft = fp * 2 + sub
                    psrc = p[:, sub * 256:(sub + 1) * 256]
                    if is_gate:
                        nc.scalar.activation(dst[:, ft], psrc, AF.Sigmoid,
                                             bias=bcol[:, ft:ft + 1])
                    elif dst is kT:
                        nc.scalar.activation(dst[:, ft], psrc, AF.Identity,
                                             bias=bcol[:, ft:ft + 1])
                    else:
                        nc.vector.tensor_scalar_add(dst[:, ft], psrc,
                                                    bcol[:, ft:ft + 1])

        v_sb = work.tile([128, 2, DI], BF16, tag="v", bufs=RB, name="v_sb")
        for tt in range(2):
            pv = pst([128, 512], name="pv")
            nc.tensor.matmul(pv, ones_1x128, bwv_row, start=True, stop=False)
            for kt in range(2):
                nc.tensor.matmul(pv, xcT[:, kt, tt * 128:(tt + 1) * 128],
                                 wv_g[:, kt], start=False, stop=(kt == 1))
            nc.vector.tensor_copy(v_sb[:, tt], pv)

        row_tiles[r] = (qT, kT, gT, v_sb)

    # ---- per-row: attention + output projection ----
    def emit_attn(r):
        qT, kT, gT, v_sb = row_tiles.pop(r)
        gatedT = work.tile([128, 4, N], BF16, tag="gatedT", bufs=3, name="gatedT")
        for pair in range(HEADS // 2):
            h0 = 2 * pair
            pTs = []
            for h in (h0, h0 + 1):
                ph, ft = (h % 2) * 64, h // 2
                s_ps = pst([128, 512], name="s_ps")
                for jt in range(2):
                    nc.tensor.matmul(s_ps[:, jt * 256:(jt + 1) * 256], ident_bf,
                                     biasT_sb[:, h * 2 + jt],
                                     start=True, stop=False)
                    nc.tensor.matmul(s_ps[:, jt * 256:(jt + 1) * 256],
                                     kT[ph:ph + 64, ft, jt * 128:(jt + 1) * 128],
                                     qT[ph:ph + 64, ft],
                                     start=False, stop=True)
                pT = work.tile([128, 512], BF16, tag="pT", bufs=2 * RB, name="pT")
                nc.scalar.activation(pT, s_ps, AF.Exp)
                pTs.append(pT)

            # Z^T: [i on partitions] via N=1 matmuls over the j tiles
            zt = pst([128, 4], F32, name="zt")
            for idx in range(2):
                for it in range(2):
                    for jt in range(2):
                        nc.tensor.matmul(
                            zt[:, idx * 2 + it:idx * 2 + it + 1],
                            pTs[idx][:, jt * 256 + it * 128:jt * 256 + (it + 1) * 128],
                            ones_col, start=(jt == 0), stop=(jt == 1))
            recipT = small.tile([128, 4], BF16, tag="recipT", name="recipT")
            nc.vector.reciprocal(recipT, zt)

            av = pst([64, 512], name="av")
            for idx, h in enumerate((h0, h0 + 1)):
                for jt in range(2):
                    nc.tensor.matmul(av[:, idx * 256:(idx + 1) * 256],
                                     v_sb[:, jt, h * DH:(h + 1) * DH],
                                     pTs[idx][:, jt * 256:(jt + 1) * 256],
                                     start=(jt == 0), stop=(jt == 1))
            # 1/Z broadcast: replicate each recipT column 64-wide on DVE, then
            # PE-transpose the block so every output row is 1/Z for that i range
            rwide = small.tile([128, 4, 64], BF16, tag="rwide", name="rwide")
            nc.vector.tensor_copy(rwide, recipT[:, :, None].to_broadcast([128, 4, 64]))
            bc = pst([64, 512], BF16, name="bc")
            for c in range(4):
                nc.tensor.transpose(bc[:, c * 128:(c + 1) * 128],
                                    rwide[:, c], ident_bf)
            bcg = work.tile([64, 512], F32, tag="bcg", bufs=3, name="bcg")
            for idx, h in enumerate((h0, h0 + 1)):
                ph, ft = (h % 2) * 64, h // 2
                sl = slice(idx * 256, (idx + 1) * 256)
                nc.vector.tensor_tensor(bcg[:, sl], bc[:, sl],
                                        gT[ph:ph + 64, ft], ALU.mult)
                nc.vector.tensor_tensor(gatedT[ph:ph + 64, ft],
                                        av[:, sl], bcg[:, sl], ALU.mult)

        pf = pst([128, 512], name="pf")
        for tt in range(2):
            nc.tensor.matmul(pf[:, tt * 256:(tt + 1) * 256], ones_1x128, bo_sb,
                             start=True, stop=False)
            for kt in range(4):
                nc.tensor.matmul(pf[:, tt * 256:(tt + 1) * 256],
                                 gatedT[:, kt, tt * 128:(tt + 1) * 128],
                                 wo_sb[:, kt], start=False, stop=(kt == 3))
        fout = work.tile([128, 512], F32, tag="fout", bufs=3, name="fout")
        nc.scalar.copy(fout, pf)
        nc.sync.dma_start(io["out"][r * N:(r + 1) * N].rearrange("(t p) d -> p t d", p=128),
                          fout.rearrange("p (t d) -> p t d", t=2))

    # ---- interleaved emission ----
    for r in range(S_PER_CORE + ATT_LAG):
        if r < S_PER_CORE:
            if r < 4:
                for c in range(8 * r, 8 * r + 8):
                    emit_bias_chunk(c)
            emit_proj(r)
            if r == 3:
                emit_bias_backs()
        if r >= ATT_LAG:
            emit_attn(r - ATT_LAG)

    ctx.close()


_NC_CACHE = {}


def _get_nc():
    if "nc" not in _NC_CACHE:
        _NC_CACHE["nc"] = build_nc()
    return _NC_CACHE["nc"]


def make_in_maps(x, edges, mask, gamma, beta, Wq, Wkv, Wo, bo, Wg, bg, Web):
    f32 = np.float32
    bf16 = ml_dtypes.bfloat16
    edgesT = np.ascontiguousarray(
        edges[0].transpose(1, 0, 2).reshape(T_EDGE, DE).T).astype(bf16)
    consts = np.concatenate(
        [np.eye(128, dtype=f32), np.ones((128, 160), f32)], axis=1).astype(bf16)
    shared = {
        "edgesT": edgesT,
        "Wq": np.ascontiguousarray(Wq, f32),
        "Wkv": np.ascontiguousarray(Wkv, f32),
        "Wg": np.ascontiguousarray(Wg, f32),
        "Wo": np.ascontiguousarray(Wo, f32),
        "Web": np.concatenate([np.asarray(Web, f32),
                               np.zeros((DE, 64 - HEADS), f32)], axis=1).astype(bf16),
        "gamma": np.asarray(gamma, f32).reshape(1, D),
        "beta": np.asarray(beta, f32).reshape(1, D),
        "bo": np.asarray(bo, f32).reshape(1, D).astype(bf16),
        "bg": np.asarray(bg, f32).reshape(1, DI),
        "consts": consts,
    }
    x0 = np.asarray(x, f32)[0]   # [S, N, D]
    in_maps = []
    for c in range(N_CORES):
        xs = np.ascontiguousarray(
            x0[c * S_PER_CORE:(c + 1) * S_PER_CORE].reshape(S_PER_CORE * N, D))
        in_maps.append({"x": xs, **shared})
    return in_maps


def kernel(x, edges, mask, gamma, beta, Wq, Wkv, Wo, bo, Wg, bg, Web,
           **run_kwargs):
    nc = _get_nc()
    in_maps = make_in_maps(x, edges, mask, gamma, beta, Wq, Wkv, Wo, bo, Wg, bg, Web)
    res = run_bass_kernel_spmd(nc, in_maps, core_ids=list(range(N_CORES)),
                               **run_kwargs)
    outs = [res.results[c]["out"].reshape(S_PER_CORE, N, D) for c in range(N_CORES)]
    full = np.concatenate(outs, axis=0)[None]   # [1, S, N, D]
    if run_kwargs:
        kernel.last_results = res
    return full
